# revision 30
# baseline (speedup 1.0000x reference)
"""Trainium2 Bass kernel for NeuralVMEmbedding (embedding lookup + VM channel injection).

Strategy (pure data-parallel over batch, bf16 internal precision):
  - 8 cores, 4 batch rows each (rows of 8192 tokens), token s -> partition
    s//64 (p-major: 64-token contiguous runs per partition).
  - bf16 table + bf16 output (host upcasts to f32; tolerance is 2e-2 vs
    bf16's ~2e-3 rounding) halves every DMA stream vs f32.
  - Two gather paths, split per 1024-token tile to balance engines:
      * DMA path: gpsimd indirect DMA per 128-token column (~1.29us of
        Q7 SWDGE descriptor-generation each).
      * PE path: one-hot matmul gather from an SBUF-resident bf16 table.
        Per tile: DVE builds a vocab-major one-hot [272, 8 cols, 128 tok]
        in one tensor_tensor (both operands unit-stride bf16 for 2x mode),
        then per column 3 accumulating matmuls against table chunks
        [<=128, 512] produce rows in PSUM; ACT drains 2 columns per copy.
  - Scan pipeline (CODE_START cummax / first CODE_END / ADDR_KEY one-hot /
    MEM mask): DVE tensor_tensor_scan per 64-token chunk + cross-partition
    exclusive-max combine via a tiny DRAM round-trip transpose. The
    ADDR_KEY mask is folded into the nibble values (masked-out tokens get
    nibble+100 so no iota value matches), so injection per tile is one
    is_equal tensor_tensor + two copy_predicated.
  - Output written with 8KB-contiguous DMA rows via the sync HWDGE queue.
"""

import sys
import numpy as np

for _p in ("/opt/trn_rl_repo",):
    if _p not in sys.path:
        sys.path.insert(0, _p)

# ---- problem constants (hardcoded per contract) ----
B, S, D, V = 32, 8192, 512, 272
NCORES = 8
RPC = B // NCORES          # rows (batch) per core = 4
P = 128                    # partitions
CPR = S // P               # columns per row in partition-major layout = 64
CTILE = 8                  # tile width in columns (CTILE*128 tokens/tile)
NT = CPR // CTILE          # tiles per row = 8
ADDR_KEY = 206
MEM_STORE = 455
# chunk 2 is padded to K=128 (table rows 16.. are zero, one-hot ids for
# p>=16 can never match a shifted token) — a K=16 matmul keeps the PE HAM
# clock gate at 1.2 GHz; full-K matmuls let it ramp to 2.4 GHz.
VCHUNKS = [(0, 128), (128, 128), (256, 128)]
# tiles (of 32) routed to the indirect-DMA gather path; the rest use the
# PE one-hot-matmul path. Balance: PE col ~660ns warm (3 matmuls) vs DMA
# col ~1.3us Pool + 366ns HBM-read; HBM writes floor ~94us/core. Kept out
# of the first tiles (PE runway while the DVE scan prelude runs) and the
# last tiles (no gather/store tail after the PE finishes).
DMA_TILES = frozenset({2, 5, 8, 11, 14, 17, 20, 23, 26})
NREP = 8   # DRAM gather-table replicas (spread HBM bank conflicts)

_CACHE = {}


def _build(mhe: int):
    from concourse import bass, bacc, mybir, tile

    f32 = mybir.dt.float32
    bf16 = mybir.dt.bfloat16
    i32 = mybir.dt.int32
    u8 = mybir.dt.uint8
    Alu = mybir.AluOpType

    nc = bacc.Bacc(None)
    tok_d = nc.declare_dram_parameter("tok", [RPC, S], i32, isOutput=False)
    # tokT[r, c, 0, p] = tok[r, p*64+c] - 64 (shift keeps 0..271 exact in
    # bf16; the singleton axis becomes the broadcast slot for the 3 vocab
    # chunks in the one-hot build)
    tokt_d = nc.declare_dram_parameter("tokt", [RPC, CPR, 1, P], bf16,
                                       isOutput=False)
    tab_d = nc.declare_dram_parameter("table", [NREP * V, D], bf16,
                                      isOutput=False)
    out_d = nc.declare_dram_parameter("out", [RPC, S, D], bf16, isOutput=True)

    with tile.TileContext(nc) as tc:
        with tc.tile_pool(name="const", bufs=1) as constp, \
             tc.tile_pool(name="pre", bufs=1) as pre, \
             tc.tile_pool(name="tokbcp", bufs=2) as tokbcp, \
             tc.tile_pool(name="dramp", bufs=1, space="DRAM") as dramp, \
             tc.tile_pool(name="mainp", bufs=8) as mainp, \
             tc.tile_pool(name="dmaxp", bufs=3) as dmaxp, \
             tc.tile_pool(name="condp", bufs=4) as condp, \
             tc.tile_pool(name="ohtp", bufs=6) as ohtp, \
             tc.tile_pool(name="poutp", bufs=4, space="PSUM") as poutp:

            # SBUF-resident table chunks for the PE path, loaded before
            # anything else so the first matmuls start early
            tab0 = constp.tile([P, D], bf16)
            nc.sync.dma_start(out=tab0[:], in_=tab_d[0:128, :])
            tab1 = constp.tile([P, D], bf16)
            nc.sync.dma_start(out=tab1[:], in_=tab_d[128:256, :])
            # chunk 2 padded to K=128: rows 16.. are zero
            tab2 = constp.tile([P, D], bf16)
            nc.vector.memset(tab2[:], 0.0)
            nc.sync.dma_start(out=tab2[0:16, :], in_=tab_d[256:272, :])
            tabs = [tab0, tab1, tab2]

            # ---------------- constants ----------------
            iota48_i = constp.tile([P, CTILE, 3, 16], i32)
            nc.gpsimd.iota(iota48_i[:], pattern=[[0, CTILE], [0, 3], [1, 16]],
                           base=0, channel_multiplier=0)
            iota48 = constp.tile([P, CTILE, 3, 16], bf16)
            nc.vector.tensor_copy(iota48[:], iota48_i[:])

            ones48 = constp.tile([P, CTILE, 64], bf16)
            nc.vector.memset(ones48[:], 1.0)

            pos_i = constp.tile([P, RPC, CPR], i32)   # pos = 64*p + c (per row)
            nc.gpsimd.iota(pos_i[:], pattern=[[0, RPC], [1, CPR]], base=0,
                           channel_multiplier=CPR)
            pos_f = constp.tile([P, RPC, CPR], f32)
            nc.vector.tensor_copy(pos_f[:], pos_i[:])

            # per-partition vocab-id rows (shifted by -64) for the three
            # one-hot chunks, materialized along the token axis so the
            # one-hot is_equal has unit-stride operands (DVE 2x mode):
            # iocE[v, 0, ci, t] = VCHUNKS[ci].lo + v - 64
            ioc_i = constp.tile([P, 1], i32)
            nc.gpsimd.iota(ioc_i[:], pattern=[[0, 1]], base=0,
                           channel_multiplier=1)
            ioc_f = constp.tile([P, 1], f32)
            nc.vector.tensor_copy(ioc_f[:], ioc_i[:])
            iocEf = constp.tile([P, 1, 3, P], f32)
            for ci, (vlo, _vw) in enumerate(VCHUNKS):
                nc.vector.tensor_scalar(iocEf[:, 0, ci, :],
                                        ioc_f[:].to_broadcast([P, P]),
                                        float(vlo - 64), None, Alu.add)
            iocE = constp.tile([P, 1, 3, P], bf16)
            nc.vector.tensor_copy(iocE[:], iocEf[:])

            # ---------------- token load ----------------
            tok_i = pre.tile([P, RPC, CPR], i32)
            nc.sync.dma_start(out=tok_i[:],
                              in_=tok_d[:].rearrange("r (p c) -> p r c", p=P))
            tok_f = pre.tile([P, RPC, CPR], f32)
            nc.vector.tensor_copy(tok_f[:], tok_i[:])

            # gather offsets with per-column table replica: c mod NREP
            crep = pre.tile([P, RPC, CPR], i32)
            nc.gpsimd.iota(crep[:], pattern=[[0, RPC], [1, CPR]], base=0,
                           channel_multiplier=0)
            nc.vector.tensor_scalar(crep[:], crep[:], NREP - 1, None,
                                    Alu.bitwise_and)
            nc.vector.tensor_scalar(crep[:], crep[:], V, None, Alu.mult)
            tokrep = pre.tile([P, RPC, CPR], i32)
            nc.vector.tensor_tensor(tokrep[:], tok_i[:], crep[:], Alu.add)

            # shifted-token broadcasts, one rotating buffer per row:
            # tokbc[p, c, 0, :] = tokt[r, c, :] on all partitions
            tokbc = {}

            def get_tokbc(r):
                if r not in tokbc:
                    tbc = tokbcp.tile([P, CPR, 1, P], bf16, tag="tokbc",
                                      name=f"tokbc{r}")
                    nc.sync.dma_start(
                        out=tbc[:],
                        in_=tokt_d[r].partition_broadcast(P))
                    tokbc[r] = tbc
                return tokbc[r]

            # -------- software-pipelined gather stage --------
            TILES = [(r, t) for r in range(RPC) for t in range(NT)]
            WARM = 6

            def emit_gather(rt):
                r, t = rt
                c0 = t * CTILE
                if r * NT + t in DMA_TILES:
                    # own x pool: keeps the Pool gather stream decoupled
                    # from the PE tiles' x rotation (whose WAR waits chain
                    # back through inject->scans)
                    x = dmaxp.tile([P, CTILE, D], bf16, tag="xd")
                    # indirect gather, one column (128 tokens) per instr
                    for k in range(CTILE):
                        nc.gpsimd.indirect_dma_start(
                            out=x[:, k, :],
                            out_offset=None,
                            in_=tab_d[:],
                            in_offset=bass.IndirectOffsetOnAxis(
                                ap=tokrep[:, r, c0 + k:c0 + k + 1], axis=0),
                        )
                else:
                    x = mainp.tile([P, CTILE, D], bf16, tag="x")
                    # one-hot matmul gather from the SBUF-resident table
                    tbc = get_tokbc(r)
                    ohT = ohtp.tile([P, CTILE, 3, P], bf16, tag="ohT")
                    nc.vector.tensor_tensor(
                        ohT[:],
                        tbc[:, c0:c0 + CTILE, :, :]
                        .to_broadcast([P, CTILE, 3, P]),
                        iocE[:].to_broadcast([P, CTILE, 3, P]),
                        Alu.is_equal)
                    for k0 in range(0, CTILE, 2):
                        pout = poutp.tile([P, 2, D], f32, tag="pout")
                        for j in range(2):
                            for ci, (vlo, vw) in enumerate(VCHUNKS):
                                nc.tensor.matmul(pout[:, j, :],
                                                 ohT[0:vw, k0 + j, ci, :],
                                                 tabs[ci][:],
                                                 start=(ci == 0),
                                                 stop=(ci == 2))
                        nc.scalar.copy(x[:, k0:k0 + 2, :], pout[:])
                return x

            xq = [emit_gather(TILES[i]) for i in range(WARM)]

            # ---------------- scan inputs ----------------
            posp1 = pre.tile([P, RPC, CPR], f32)
            nc.vector.tensor_scalar(posp1[:], pos_f[:], 1.0, None, Alu.add)
            posm1 = pre.tile([P, RPC, CPR], f32)
            nc.vector.tensor_scalar(posm1[:], pos_f[:], 1.0, None, Alu.subtract)

            # v0 = (tok==256)*(pos+1) - 1   (CODE_START candidate positions)
            v0 = pre.tile([P, RPC, CPR], f32)
            nc.vector.scalar_tensor_tensor(v0[:], tok_f[:], 256.0, posp1[:],
                                           Alu.is_equal, Alu.mult)
            nc.vector.tensor_scalar(v0[:], v0[:], 1.0, None, Alu.subtract)

            # v1 = (tok==257)  (CODE_END seen)
            v1 = pre.tile([P, RPC, CPR], f32)
            nc.vector.tensor_scalar(v1[:], tok_f[:], 257.0, None, Alu.is_equal)

            cs = pre.tile([P, RPC, CPR], f32)
            ce = pre.tile([P, RPC, CPR], f32)

            # --- level 1: within-partition prefix max over 64-token chunks ---
            loc_cs = pre.tile([P, RPC, CPR], f32)
            loc_ce = pre.tile([P, RPC, CPR], f32)
            for r in range(RPC):
                nc.vector.tensor_tensor_scan(loc_cs[:, r, :], v0[:, r, :],
                                             v0[:, r, :], -1.0,
                                             Alu.max, Alu.bypass)
                nc.vector.tensor_tensor_scan(loc_ce[:, r, :], v1[:, r, :],
                                             v1[:, r, :], 0.0,
                                             Alu.max, Alu.bypass)

            # --- level 2: exclusive prefix max across partitions (chunks) ---
            NS = 2 * RPC
            f8 = pre.tile([P, NS], f32)
            for r in range(RPC):
                nc.vector.tensor_copy(f8[:, r:r + 1],
                                      loc_cs[:, r, CPR - 1:CPR])
                nc.vector.tensor_copy(f8[:, RPC + r:RPC + r + 1],
                                      loc_ce[:, r, CPR - 1:CPR])
            f8_d = dramp.tile([P, NS], f32)
            nc.sync.dma_start(out=f8_d[:], in_=f8[:])
            f8t = pre.tile([NS, P], f32)
            nc.sync.dma_start(out=f8t[:], in_=f8_d[:].rearrange("p j -> j p"))
            p8 = pre.tile([NS, P], f32)
            nc.vector.tensor_tensor_scan(p8[:], f8t[:], f8t[:], -1e30,
                                         Alu.max, Alu.bypass)
            e8t = pre.tile([NS, P], f32)
            # -1 is a neutral carry for both scans (cs values >= -1, ce >= 0)
            nc.vector.memset(e8t[:, 0:1], -1.0)
            nc.vector.tensor_copy(e8t[:, 1:P], p8[:, 0:P - 1])
            e8_d = dramp.tile([NS, P], f32)
            nc.sync.dma_start(out=e8_d[:], in_=e8t[:])
            e8 = pre.tile([P, NS], f32)
            nc.sync.dma_start(out=e8[:], in_=e8_d[:].rearrange("j p -> p j"))

            # --- combine ---
            nc.vector.tensor_tensor(cs[:], loc_cs[:],
                                    e8[:, 0:RPC].to_broadcast([P, RPC, CPR]),
                                    Alu.max)
            nc.vector.tensor_tensor(ce[:], loc_ce[:],
                                    e8[:, RPC:NS].to_broadcast([P, RPC, CPR]),
                                    Alu.max)

            # ---------------- per-token derived values ----------------
            # mask = (cs >= 0) & (ce == 0) & (tok < 256)
            m3 = pre.tile([P, RPC, CPR], f32)
            nc.vector.tensor_scalar(m3[:], tok_f[:], 255.5, None, Alu.is_lt)
            m23 = pre.tile([P, RPC, CPR], f32)
            nc.vector.scalar_tensor_tensor(m23[:], ce[:], 0.5, m3[:],
                                           Alu.is_lt, Alu.mult)
            mask = pre.tile([P, RPC, CPR], f32)
            nc.vector.scalar_tensor_tensor(mask[:], cs[:], 0.0, m23[:],
                                           Alu.is_ge, Alu.mult)
            # pen = 100 where the ADDR injection is masked OFF (so the
            # penalized nibble never equals any iota value 0..15)
            pen = pre.tile([P, RPC, CPR], f32)
            nc.vector.tensor_scalar(pen[:], mask[:], 0.5, 100.0,
                                    Alu.is_lt, Alu.mult)

            # seq_pos = max(pos - 1 - cs, 0)
            sp = pre.tile([P, RPC, CPR], f32)
            nc.vector.scalar_tensor_tensor(sp[:], cs[:], -1.0, posm1[:],
                                           Alu.mult, Alu.add)
            nc.vector.tensor_scalar(sp[:], sp[:], 0.0, None, Alu.max)

            # q = floor(sp / 5), robust to cast rounding mode:
            #   y = sp*0.2 ; q0 = int(y) ; q = q0 - (y - float(q0) < 0)
            y = pre.tile([P, RPC, CPR], f32)
            nc.vector.tensor_scalar(y[:], sp[:], 0.2, None, Alu.mult)
            q_i = pre.tile([P, RPC, CPR], i32)
            nc.vector.tensor_copy(q_i[:], y[:])
            q_f = pre.tile([P, RPC, CPR], f32)
            nc.vector.tensor_copy(q_f[:], q_i[:])
            corr = pre.tile([P, RPC, CPR], f32)
            nc.vector.tensor_tensor(corr[:], y[:], q_f[:], Alu.subtract)
            nc.vector.tensor_scalar(corr[:], corr[:], 0.0, None, Alu.is_lt)
            nc.vector.tensor_tensor(q_f[:], q_f[:], corr[:], Alu.subtract)

            # addr = sp + 3*q  (int32)
            sp_i = pre.tile([P, RPC, CPR], i32)
            nc.vector.tensor_copy(sp_i[:], sp[:])
            q_i2 = pre.tile([P, RPC, CPR], i32)
            nc.vector.tensor_copy(q_i2[:], q_f[:])
            q3 = pre.tile([P, RPC, CPR], i32)
            nc.vector.tensor_scalar(q3[:], q_i2[:], 1, None,
                                    Alu.logical_shift_left)
            nc.vector.tensor_tensor(q3[:], q3[:], q_i2[:], Alu.add)
            addr = pre.tile([P, RPC, CPR], i32)
            nc.vector.tensor_tensor(addr[:], sp_i[:], q3[:], Alu.add)

            # masked nibbles, packed [P, RPC, CPR, 3] and cast to bf16:
            # mnib[..., b] = nib_b + pen
            lo_i = pre.tile([P, RPC, CPR], i32)
            nc.vector.tensor_scalar(lo_i[:], addr[:], 15, None, Alu.bitwise_and)
            hi_i = pre.tile([P, RPC, CPR], i32)
            nc.vector.tensor_scalar(hi_i[:], addr[:], 4, 15,
                                    Alu.logical_shift_right, Alu.bitwise_and)
            top_i = pre.tile([P, RPC, CPR], i32)
            nc.vector.tensor_scalar(top_i[:], addr[:], 8, 15,
                                    Alu.logical_shift_right, Alu.bitwise_and)
            mnibf = pre.tile([P, RPC, CPR, 3], f32)
            for b, nib_i in enumerate((lo_i, hi_i, top_i)):
                nib_f = pre.tile([P, RPC, CPR], f32, name=f"nib_f{b}")
                nc.vector.tensor_copy(nib_f[:], nib_i[:])
                nc.vector.tensor_tensor(mnibf[:, :, :, b], nib_f[:], pen[:],
                                        Alu.add)
            mnib = pre.tile([P, RPC, CPR, 3], bf16)
            nc.vector.tensor_copy(mnib[:], mnibf[:])

            # cond2 = (tok == 258) & (pos < mem_history_end)
            m5 = pre.tile([P, RPC, CPR], f32)
            nc.vector.tensor_scalar(m5[:], pos_f[:], float(mhe), None, Alu.is_lt)
            cond2 = pre.tile([P, RPC, CPR], u8)
            nc.vector.scalar_tensor_tensor(cond2[:], tok_f[:], 258.0, m5[:],
                                           Alu.is_equal, Alu.mult)

            # ---------------- pipelined inject + store loop ----------------
            out_v = out_d[:].rearrange("r (p c) d -> r p c d", p=P)
            for i, (r, t) in enumerate(TILES):
                if i + WARM < len(TILES):
                    xq.append(emit_gather(TILES[i + WARM]))
                x = xq[i]
                c0 = t * CTILE
                csl = slice(c0, c0 + CTILE)
                cond = condp.tile([P, CTILE, 64], u8, tag="cond")
                nc.vector.tensor_tensor(
                    cond[:, :, 0:48].rearrange("p c (a b) -> p c a b", b=16),
                    iota48[:],
                    mnib[:, r, csl, :].to_broadcast([P, CTILE, 3, 16]),
                    Alu.is_equal)
                nc.vector.copy_predicated(
                    out=x[:, :, ADDR_KEY:ADDR_KEY + 48],
                    mask=cond[:, :, 0:48],
                    data=ones48[:, :, 0:48])
                nc.vector.copy_predicated(
                    out=x[:, :, MEM_STORE],
                    mask=cond2[:, r, csl], data=ones48[:, :, 0])
                nc.sync.dma_start(out=out_v[r, :, csl, :], in_=x[:])
    nc.finalize()
    return nc


def _get_nc(mhe: int):
    if mhe not in _CACHE:
        _CACHE[mhe] = _build(mhe)
    return _CACHE[mhe]


def make_in_maps(tok, tab):
    """tok: int32 [B, S]; tab: float32 [V, D] -> per-core input dicts."""
    import ml_dtypes

    tab_bf = np.ascontiguousarray(
        np.tile(tab.astype(ml_dtypes.bfloat16), (NREP, 1)))
    maps = []
    for c in range(NCORES):
        tok_c = np.ascontiguousarray(tok[c * RPC:(c + 1) * RPC])
        tokt = np.ascontiguousarray(
            (tok_c.reshape(RPC, P, CPR).transpose(0, 2, 1) - 64)
            .astype(ml_dtypes.bfloat16))[:, :, None, :]
        maps.append({"tok": tok_c, "tokt": tokt, "table": tab_bf})
    return maps


def kernel(token_ids, embed_table, mem_history_end):
    from concourse.bass_utils import run_bass_kernel_spmd

    tok = np.asarray(token_ids)
    tab = np.ascontiguousarray(np.asarray(embed_table, dtype=np.float32))
    mhe = int(mem_history_end)
    assert tok.shape == (B, S) and tab.shape == (V, D)
    tok = np.ascontiguousarray(tok.astype(np.int32, copy=False))

    nc = _get_nc(mhe)
    in_maps = make_in_maps(tok, tab)
    res = run_bass_kernel_spmd(nc, in_maps, list(range(NCORES))).results
    out = np.concatenate(
        [np.asarray(res[c]["out"]).astype(np.float32) for c in range(NCORES)],
        axis=0)
    return out.reshape(B, S, D)
